# revision 79
# baseline (speedup 1.0000x reference)
"""Trainium2 Bass kernel for conv-QK causal attention + MLP.

Reference computation (B=4, T=2048, D=512, H=8, DK=DV=64, FS=3):
  q = causal_conv1d(x, Wq) + bq ; k = causal_conv1d(x, Wk) + bk
  v = x @ Wv + bv
  per-head causal attention (softmax(q k^T / 8))
  out = relu(attn @ W1 + b1) @ W2 + b2        -> [B, T, 64]

Sharding: head-parallel, one head per NeuronCore (H == 8 == n_cores).
Each core computes q/k/v and attention for its head over all batches,
its partial attn @ W1[head], then ReduceScatters to sum head partials
and shard tokens 8-way for the final relu/W2 epilogue.

On-chip layout is "transposed" (channels on partitions, tokens on the
free axis) so softmax needs no transposes at all:
  St[k, q] = K Q^T per 128-row k-block strip; exp on ScalarE (no max
  subtraction: logits are O(1) by construction); causal masking via a
  0/1 upper-triangular multiply on the diagonal block; P V done as
  O~^T = [V | 1]^T P^T which also accumulates the softmax denominators
  as row 64 of the PSUM accumulator; softmax normalization happens
  AFTER the W1 partial (1/l applied to the W1 output on DVE).

Key perf structure (v2), driven by the HAM clock-gate trace: the PE
HAM demotes to K=4/8 (1.2 GHz) during stretches dominated by thin
64-row stationaries and any >3.4us PE idle window.  So:
  - Batch b+1's projection work (QK conv, V, V-transposes — all
    128-row stationaries) is interleaved step-by-step into batch b's
    attention phase: the PE stream stays dense and majority-full-width
    through the whole kernel, and conv/attention no longer alternate
    as ACT-bound vs PE-bound phases.
  - QK^T strips use a 128-row stationary kzt whose rows 64:128 are
    zeros (K^T lives in rows 0:64); the moving operand is the full
    128-row qkt tile (rows 64:128 are K^T, killed by the zero rows).
    Mathematically identical, but the stationary is full-height.
  - V^T staging tiles are 128 rows (rows 65:128 zeroed) so the PE
    transposes are full-height too.
  - QK^T emits per-512-column chunks into single-bank PSUM tiles
    (4-buf rotation) with exp issued per chunk: finer ACT pipelining
    and earlier PSUM release.
  - Collectives: the reduce-scatter stream is serial (~10-20 GB/s per
    op + ~6us trigger overhead, each op's duration also absorbing
    cross-core skew), so the payload is bf16 and grouped as
    [b0, b1, b2, b3h0, b3h1] — the middle ops drain while compute
    runs and only a small 128-token-shard op sits in the kernel tail.
    The FIRST collective's trigger would wait for the startup barrier
    on the gpsimd queue (stalling the close-chain partition_broadcasts
    behind it, measured -22us), so it is deferred a few iterations.
  - The epilogue is split: batches 0-2 in one fused pass that overlaps
    the compute drain, then each batch-3 half as its own small chain
    the moment its collective lands.

dtypes: the whole heavy path runs in bf16; PSUM accumulation and the
softmax denominators stay fp32.  The PE queue is strictly in-order, so
anything that waits on ACT/DVE/collective latency is emitted a few
pair-iterations after the data chain that feeds it (the `pending`
deferral list) or it stalls every matmul behind it.
"""

import ml_dtypes
import numpy as np

import concourse.bass as bass
import concourse.mybir as mybir
import concourse.tile as tile
from concourse import bacc, bass_utils
from concourse.masks import make_identity, make_upper_triangular

B, T, D = 4, 2048, 512
H, DK, DV, FS = 8, 64, 64, 3
NCORES = 8
TP = T + FS - 1          # left-zero-padded time axis (2050)
NDT = D // 128           # d-tiles (4)
NTT = T // 128           # t-tiles (16)
TOK = B * T // NCORES    # 1024 output tokens per core
NCH = T // 512           # 512-col conv chunks per batch (4)

F32 = mybir.dt.float32
BF16 = mybir.dt.bfloat16

_STATE = {}


def _build():
    nc = bacc.Bacc("TRN2", target_bir_lowering=False, debug=False,
                   num_devices=NCORES)

    xtp = nc.dram_tensor("xtp", [B, D, TP], BF16, kind="ExternalInput")
    wqk = nc.dram_tensor("wqk", [FS, NDT, 128, 128], BF16, kind="ExternalInput")
    wv = nc.dram_tensor("wv", [NDT, 128, DV], BF16, kind="ExternalInput")
    w1 = nc.dram_tensor("w1", [DV, 64], BF16, kind="ExternalInput")
    w2 = nc.dram_tensor("w2", [64, 64], BF16, kind="ExternalInput")
    bqk = nc.dram_tensor("bqk", [128, 1], F32, kind="ExternalInput")
    bv = nc.dram_tensor("bv", [64, 1], F32, kind="ExternalInput")
    b1 = nc.dram_tensor("b1", [64, 1], F32, kind="ExternalInput")
    b2 = nc.dram_tensor("b2", [64, 1], F32, kind="ExternalInput")
    out = nc.dram_tensor("out", [DV, TOK], BF16, kind="ExternalOutput")

    EXP = mybir.ActivationFunctionType.Exp

    with tile.TileContext(nc) as tc:
        with (
            tc.tile_pool(name="cpool", bufs=1) as cpool,
            tc.tile_pool(name="xpool", bufs=2) as xpool,
            tc.tile_pool(name="qkpool", bufs=2) as qkpool,
            tc.tile_pool(name="vpool", bufs=2) as vpool,
            tc.tile_pool(name="ptpool", bufs=6) as ptpool,
            tc.tile_pool(name="atpool", bufs=2) as atpool,
            tc.tile_pool(name="spool", bufs=2) as spool,
            tc.tile_pool(name="stpool", bufs=4, space="PSUM") as stpool,
            tc.tile_pool(name="opool", bufs=2, space="PSUM") as opool,
            tc.tile_pool(name="cvpool", bufs=2, space="PSUM") as cvpool,
            tc.tile_pool(name="dpool", bufs=1, space="DRAM") as dpool,
        ):
            xtp_tiles = {}

            def load_xtp(b):
                # one tile PER d-quarter: readers then depend on exactly
                # the quarter's DMA, not conservatively on all four
                # sequential ring transfers (~13us for the whole load)
                src = xtp.ap()[b].rearrange("(dt p) t -> p dt t", p=128)
                qs_ = []
                for dt_ in range(NDT):
                    t_ = xpool.tile([128, TP], BF16,
                                    name=f"xtp_sb{b}_{dt_}",
                                    tag=f"xtp{dt_}")
                    # scalar-engine hwdge ring: keeps the 525KB x loads
                    # off the sync ring (rs_in/zin/consts traffic)
                    nc.scalar.dma_start(t_[:], src[:, dt_])
                    qs_.append(t_)
                xtp_tiles[b] = qs_

            # ---- wqk then x batch 0 first (warmup needs wqk, the first
            # conv matmuls need both); everything else loads behind them ----
            wqk_sb = cpool.tile([128, FS, NDT, 128], BF16)
            nc.sync.dma_start(wqk_sb[:], wqk.ap().rearrange("f dt p m -> p f dt m"))
            load_xtp(0)
            _deferred_consts = []
            _orig_dma = nc.sync.dma_start

            def _defer_dma(*a, **k):
                _deferred_consts.append((a, k))
            wv_sb = cpool.tile([128, NDT, DV], BF16)
            _defer_dma(wv_sb[:], wv.ap().rearrange("dt p m -> p dt m"))
            w1_sb = cpool.tile([DV, 64], BF16)
            _defer_dma(w1_sb[:], w1.ap())
            w2_sb = cpool.tile([64, 64], BF16)
            _defer_dma(w2_sb[:], w2.ap())
            bqk_sb = cpool.tile([128, 1], F32)
            _defer_dma(bqk_sb[:], bqk.ap())
            bv_sb = cpool.tile([64, 1], F32)
            _defer_dma(bv_sb[:], bv.ap())
            b1_sb = cpool.tile([64, 1], F32)
            _defer_dma(b1_sb[:], b1.ap())
            b2_sb = cpool.tile([64, 1], F32)
            _defer_dma(b2_sb[:], b2.ap())
            maskf_sb = cpool.tile([128, 128], F32)
            make_upper_triangular(nc, maskf_sb[:], val=1.0, diag=True)
            mask_sb = cpool.tile([128, 128], BF16)
            nc.vector.tensor_copy(mask_sb[:], maskf_sb[:])
            # touch Exp once so the ~2.7us ACT table load happens during
            # startup instead of stalling the first attention strip
            junk_sb = cpool.tile([1, 128], F32)
            nc.scalar.activation(junk_sb[:], maskf_sb[0:1, :],
                                 mybir.ActivationFunctionType.Exp, scale=1.0)
            # persistent V^T staging tiles: 128 rows so the PE transposes
            # are full-height; row 64 is the ones row ([V | 1] denominator
            # column), rows 65:128 are zeros
            vt_tiles = [cpool.tile([128, T], BF16, name=f"vt_t{i}")
                        for i in range(2)]
            for t_ in vt_tiles:
                nc.gpsimd.memset(t_[64:128, :].bitcast(mybir.dt.uint16), 0)
                nc.gpsimd.memset(t_[64:65, :].bitcast(mybir.dt.uint16),
                                 0x3F80)  # bf16 1.0
            # persistent K^T stationary tiles: rows 0:64 = K^T (written per
            # batch by the conv drains), rows 64:128 = zeros, so QK^T loads
            # a full 128-row stationary (HAM stays warm) while the junk
            # K^T rows of the moving qkt operand are multiplied by zero
            kzt_tiles = [cpool.tile([128, T], BF16, name=f"kzt_t{i}")
                         for i in range(2)]
            for t_ in kzt_tiles:
                nc.gpsimd.memset(t_[64:128, :].bitcast(mybir.dt.uint16), 0)
            identf_sb = cpool.tile([128, 128], F32)
            make_identity(nc, identf_sb[:])
            ident_sb = cpool.tile([128, 128], BF16)
            nc.vector.tensor_copy(ident_sb[:], identf_sb[:])

            # The collective stream is serial, ~10-20 GB/s per op plus
            # ~6us trigger overhead, and can only start once the first
            # op's inputs exist — so lead with two small half-batch ops
            # (ready earliest), batch the middle, and end with two small
            # ops so only ~9us of collective sits in the kernel tail.
            RSQ = [256, 256, 256, 128, 128]  # shard tokens per op
            RSTB = [0, 2048, 4096, 6144, 7168]  # first token per op
            NRS = len(RSQ)
            rs_in = [dpool.tile([NCORES, 64, RSQ[i]], BF16, name=f"rs_in{i}",
                                tag=f"rsi{i}", uniquify=False)
                     for i in range(NRS)]
            rs_out = [dpool.tile([64, RSQ[i]], BF16, name=f"rs_out{i}",
                                 tag=f"rso{i}", uniquify=False)
                      for i in range(NRS)]
            rs_left = {0: 4, 1: 4, 2: 4, 3: 2, 4: 2}

            # The PE queue is strictly in-order, so a matmul that depends on
            # the (ACT/DMA/DVE) softmax-normalization chain must be emitted a
            # couple of pair-iterations later than the chain or it stalls the
            # whole queue.  pending holds [age, closure, threshold] entries.
            pending = []

            def flush_pending(extra_age):
                keep = []
                for item in pending:
                    if item[0] + extra_age >= item[2]:
                        item[1]()
                    else:
                        keep.append(item)
                pending[:] = keep

            def age_pending():
                for item in pending:
                    item[0] += 1

            # ---- projection work for batch b, as a list of fine-grained
            # steps (~0.5-1us of PE work each) interleaved into batch b-1's
            # attention phase.  All stationaries are full 128-row. ----
            prep = {}

            def make_conv_steps(b):
                xsb = xtp_tiles.pop(b)
                qkt = qkpool.tile([128, T], BF16, name=f"qkt{b}", tag="qkt")
                kzt = kzt_tiles[b % 2]
                vt = vt_tiles[b % 2]
                vsb = vpool.tile([128, NTT, 80], BF16, name=f"v_sb{b}",
                                 tag="v")
                prep[b] = (qkt, vsb)
                steps = []

                # chunk PAIRS share each stationary (consecutive matmuls
                # elide the redundant LDWEIGHTS: 24 loads/batch not 48);
                # dt outer so the first steps only need the first xtp
                # quarter (the conv can start while the rest still loads)
                def qk_step(cp, dt_, f, qkcs):
                    def run():
                        for ci in range(2):
                            c = 2 * cp + ci
                            nc.tensor.matmul(
                                qkcs[ci][:],
                                wqk_sb[:, f, dt_, :],
                                xsb[dt_][:, c * 512 + f: c * 512 + f + 512],
                                start=(dt_ == 0 and f == 0),
                                stop=(dt_ == NDT - 1 and f == FS - 1))
                        if dt_ == NDT - 1 and f == FS - 1:
                            for ci in range(2):
                                sl = bass.ts(2 * cp + ci, 512)
                                nc.vector.tensor_scalar_add(
                                    qkt[:, sl], qkcs[ci][:], bqk_sb[:])
                                nc.vector.tensor_copy(
                                    kzt[0:64, sl], qkt[64:128, sl])
                    return run

                for cp in range(NCH // 2):
                    qkcs = [cvpool.tile([128, 512], F32, tag="cv",
                                        name=f"qkc{b}_{2 * cp + ci}")
                            for ci in range(2)]
                    for dt_ in range(NDT):
                        for f in range(FS):
                            steps.append(qk_step(cp, dt_, f, qkcs))

                def v_step(cp, dt_, vcs):
                    def run():
                        for ci in range(2):
                            c = 2 * cp + ci
                            nc.tensor.matmul(
                                vcs[ci][0:64, :],
                                wv_sb[:, dt_, :],
                                xsb[dt_][:, c * 512 + 2: c * 512 + 2 + 512],
                                start=(dt_ == 0), stop=(dt_ == NDT - 1))
                        if dt_ == NDT - 1:
                            for ci in range(2):
                                nc.vector.tensor_scalar_add(
                                    vt[0:64, bass.ts(2 * cp + ci, 512)],
                                    vcs[ci][0:64, :], bv_sb[:])
                    return run

                def tr_step(c):
                    def run():
                        for tt in range(4 * c, 4 * c + 4):
                            trp = cvpool.tile([128, 128], BF16, tag="cv",
                                              name=f"trp{b}_{tt}")
                            nc.tensor.transpose(
                                trp[:, 0:80], vt[:, bass.ts(tt, 128)],
                                ident_sb[:, 0:80])
                            nc.vector.tensor_copy(vsb[:, tt, :],
                                                  trp[:, 0:80])
                    return run

                for cp in range(NCH // 2):
                    vcs = [cvpool.tile([128, 512], F32, tag="cv",
                                       name=f"vc{b}_{2 * cp + ci}")
                           for ci in range(2)]
                    for dt_ in range(NDT):
                        steps.append(v_step(cp, dt_, vcs))
                    steps.append(tr_step(2 * cp))
                    steps.append(tr_step(2 * cp + 1))
                return steps

            conv_steps = []

            def take_conv(iters_left):
                if not conv_steps:
                    return
                n = -(-len(conv_steps) // max(iters_left, 1))
                for _ in range(min(n, len(conv_steps))):
                    conv_steps.pop(0)()

            # ---- PE warmup while the first DMAs land ----
            warm_ps = stpool.tile([128, 512], F32, tag="st")
            for wi in range(8):
                nc.tensor.matmul(
                    warm_ps[:], wqk_sb[:, 0, 0, :], wqk_sb[:, 0, :, 0:128],
                    start=(wi == 0), stop=(wi == 7))
            for a_, k_ in _deferred_consts:
                _orig_dma(*a_, **k_)
            _deferred_consts.clear()
            load_xtp(1)
            # the warm-result DMA waits on the warmup matmuls, so it goes
            # last in the queue (nothing behind it to block)
            warm_sb = cpool.tile([1, 1], F32)
            nc.vector.tensor_copy(warm_sb[:], warm_ps[0:1, 0:1])
            nc.sync.dma_start(out.ap()[0:1, 0:2].bitcast(F32), warm_sb[:])

            # batch 0's projections run as a dense phase (nothing to
            # interleave them into)
            for s in make_conv_steps(0):
                s()

            for b in range(B):
                qkt_sb, v_sb = prep.pop(b)
                kzt = kzt_tiles[b % 2]
                if b + 2 <= B - 1:
                    load_xtp(b + 2)
                if b + 1 <= B - 1:
                    conv_steps.extend(make_conv_steps(b + 1))
                # pair-iterations over both passes: 4+2 + 8+2
                iters_left = (1024 // 256 + 2) + (2048 // 256 + 2)

                # ---- attention, two q-passes of 1024 columns ----
                attnT_sb = atpool.tile([64, T], BF16, tag="attnT")
                # bf16: the reduce-scatter payload halves (the collective
                # stream is the kernel-tail critical path)
                p1_sb = atpool.tile([64, T], BF16, tag="p1")
                for ps in (0, 1):
                    qlo, qhi = ps * 1024, ps * 1024 + 1024
                    o_ps = [opool.tile([128, 512], F32, tag="o",
                                       name=f"o_ps_{b}_{ps}_{c}")
                            for c in range(2)]
                    nkb = qhi // 128

                    def do_pv(strips, o_ps=o_ps, qlo=qlo, b=b):
                        """PV matmuls for a pair of exp'd strips; returns the
                        chunks whose accumulation closed."""
                        closed = []
                        for kb, qs, w, pt in strips:
                            for qc in range(2):
                                clo = qlo + qc * 512
                                chi = clo + 512
                                lo = max(clo, qs)
                                n = chi - lo
                                if n <= 0:
                                    continue
                                stop = kb == chi // 128 - 1
                                nc.tensor.matmul(
                                    o_ps[qc][0:DV + 1, lo - clo: lo - clo + n],
                                    v_sb[:, kb, 0:DV + 1],
                                    pt[:, lo - qs: lo - qs + n],
                                    start=(kb == 0),
                                    stop=stop)
                                if stop:
                                    closed.append(qc)
                        return closed

                    window = []  # strip pairs awaiting PV (lag 2)
                    for kba in list(range(0, nkb, 2)) + [None, None]:
                        take_conv(iters_left)
                        iters_left -= 1
                        age_pending()
                        flush_pending(0)
                        strips = []
                        if kba is not None:
                            for kb in (kba, kba + 1):
                                qs = max(qlo, kb * 128)
                                w = qhi - qs
                                pt = ptpool.tile([128, 1024], BF16, tag="pt")
                                lhsT = kzt[:, bass.ts(kb, 128)]
                                for c0 in range(0, w, 512):
                                    cw = min(512, w - c0)
                                    stc = stpool.tile([128, 512], F32,
                                                      tag="st")
                                    nc.tensor.matmul(
                                        stc[:, 0:cw], lhsT,
                                        qkt_sb[:, qs + c0: qs + c0 + cw],
                                        start=True, stop=True)
                                    nc.scalar.activation(
                                        pt[:, c0:c0 + cw], stc[:, 0:cw],
                                        EXP, scale=0.125)
                                    if c0 == 0 and kb * 128 >= qlo:
                                        nc.vector.tensor_mul(
                                            pt[:, 0:128], pt[:, 0:128],
                                            mask_sb[:])
                                strips.append((kb, qs, w, pt))
                        # PV lags two pair-iterations so it never waits on exp
                        if strips:
                            window.append(strips)
                        closed = []
                        if len(window) == 3 or (kba is None and window):
                            closed = do_pv(window.pop(0))
                        # per-chunk tail as soon as its accumulation closes:
                        # 1/l straight off the PSUM denominator row, copy the
                        # unnormalized rows to bf16, then (deferred) W1 into
                        # partitions 64:128 of the same bank and normalize on
                        # the way out to the reduce-scatter buffer.
                        for qc in closed:
                            clo = qlo + qc * 512
                            o_t = o_ps[qc]
                            l_sb = spool.tile([1, 512], F32, tag="l")
                            nc.vector.tensor_copy(l_sb[:], o_t[DV:DV + 1, :])
                            linv_sb = spool.tile([1, 512], F32, tag="linv")
                            nc.vector.reciprocal_approx_fast(
                                linv_sb[:], l_sb[:])
                            lbc_sb = spool.tile([64, 512], F32, tag="lbc")
                            nc.gpsimd.partition_broadcast(lbc_sb[:], linv_sb[:])
                            nc.vector.tensor_copy(
                                attnT_sb[:, clo:clo + 512], o_t[0:DV, :])

                            def p1_work(clo=clo, b=b, o_t=o_t, lbc_sb=lbc_sb,
                                        attnT_sb=attnT_sb, p1_sb=p1_sb):
                                nc.tensor.matmul(
                                    o_t[64:128, :], w1_sb[:],
                                    attnT_sb[:, clo:clo + 512],
                                    start=True, stop=True)
                                nc.vector.tensor_mul(
                                    p1_sb[:, clo:clo + 512], o_t[64:128, :],
                                    lbc_sb[:])
                                idx = b if b < B - 1 else 3 + clo // 1024
                                qsh = RSQ[idx]
                                for hh in range(512 // qsh):
                                    s0 = clo + qsh * hh
                                    nc.sync.dma_start(
                                        rs_in[idx][
                                            (s0 % (NCORES * qsh)) // qsh],
                                        p1_sb[:, s0:s0 + qsh])
                                rs_left[idx] -= 1
                                if rs_left[idx] == 0:
                                    def rs_fire(idx=idx):
                                        nc.gpsimd.collective_compute(
                                            "ReduceScatter",
                                            mybir.AluOpType.add,
                                            replica_groups=[
                                                list(range(NCORES))],
                                            ins=[rs_in[idx].opt()],
                                            outs=[rs_out[idx].opt()],
                                        )
                                    if idx == 0:
                                        # the first collective's trigger
                                        # waits for the startup barrier ON
                                        # THE GPSIMD QUEUE, stalling the
                                        # close-chain broadcasts behind it
                                        # — fire it a few iterations late
                                        pending.append([0, rs_fire, 6])
                                    else:
                                        rs_fire()
                            pending.append([0, p1_work, 2])
                    # drain this pass's deferred W1/RS chains in the
                    # inter-pass slack: the next pass's first PV reuses
                    # the same two o_ps PSUM slots and would otherwise
                    # collide with them two iterations in
                    flush_pending(2)
                # any projection steps not consumed by the pair-iterations
                while conv_steps:
                    conv_steps.pop(0)()

            flush_pending(99)

            # ---- split epilogue: relu(z + b1) @ W2 + b2.  The batch-0..2
            # part overlaps the compute drain (its collectives finish while
            # batch 3 computes); each batch-3 half runs as soon as its own
            # collective lands.  out columns: [b0|b1|b2|b3h0|b3h1]. ----
            RSC0 = [0, 256, 512, 768, 896]  # out column start per op
            zin_sb = spool.tile([64, 1024], BF16)
            z_sb = spool.tile([64, 1024], BF16)
            fin_sb = spool.tile([64, 1024], BF16)

            def epi_part(ops):
                c0, c1 = RSC0[ops[0]], RSC0[ops[-1]] + RSQ[ops[-1]]
                for q in ops:
                    nc.sync.dma_start(
                        zin_sb[:, bass.ds(RSC0[q], RSQ[q])], rs_out[q][:])
                sl = bass.ds(c0, c1 - c0)
                nc.vector.tensor_scalar(
                    z_sb[:, sl], zin_sb[:, sl], b1_sb[:], 0.0,
                    op0=mybir.AluOpType.add, op1=mybir.AluOpType.max)
                for m0 in range(c0, c1, 512):
                    cw = min(512, c1 - m0)
                    f_ps = opool.tile([64, 512], F32, tag="o",
                                      name=f"f_ps{m0}")
                    nc.tensor.matmul(f_ps[0:64, 0:cw], w2_sb[:],
                                     z_sb[:, m0:m0 + cw],
                                     start=True, stop=True)
                    nc.vector.tensor_scalar_add(
                        fin_sb[:, m0:m0 + cw], f_ps[0:64, 0:cw], b2_sb[:])
                nc.sync.dma_start(out.ap()[:, sl], fin_sb[:, sl])

            epi_part([0, 1, 2])
            epi_part([3])
            epi_part([4])

    nc.compile()
    return nc


def _get_nc():
    if "nc" not in _STATE:
        _STATE["nc"] = _build()
    return _STATE["nc"]


def _prep_inputs(x, Wq, bq, Wk, bk, Wv, bv, W1, b1, W2, b2):
    f = np.float32
    x = np.ascontiguousarray(np.asarray(x, f))
    xtp = np.zeros((B, D, TP), f)
    xtp[:, :, FS - 1:] = x.transpose(0, 2, 1)
    Wq = np.asarray(Wq, f)
    Wk = np.asarray(Wk, f)
    Wv = np.asarray(Wv, f)
    W1 = np.asarray(W1, f)
    W2 = np.asarray(W2, f)
    bq = np.asarray(bq, f)
    bk = np.asarray(bk, f)
    bv = np.asarray(bv, f)
    b1 = np.asarray(b1, f)
    b2 = np.asarray(b2, f)

    bf = ml_dtypes.bfloat16
    xtp = np.ascontiguousarray(xtp.astype(bf))
    in_maps = []
    for c in range(NCORES):
        hs = slice(c * DK, (c + 1) * DK)
        wqk_c = np.concatenate([Wq[:, :, hs], Wk[:, :, hs]], axis=2)
        in_maps.append({
            "xtp": xtp,
            "wqk": np.ascontiguousarray(
                wqk_c.reshape(FS, NDT, 128, 128).astype(bf)),
            "wv": np.ascontiguousarray(Wv[:, hs].reshape(NDT, 128, DV).astype(bf)),
            "w1": np.ascontiguousarray(W1[hs, :].astype(bf)),
            "w2": np.ascontiguousarray(W2.astype(bf)),
            "bqk": np.ascontiguousarray(
                np.concatenate([bq[hs], bk[hs]])[:, None]),
            "bv": np.ascontiguousarray(bv[hs][:, None]),
            "b1": np.ascontiguousarray(b1[:, None]),
            "b2": np.ascontiguousarray(b2[:, None]),
        })
    return in_maps


def _run(inputs, trace=False):
    nc = _get_nc()
    in_maps = _prep_inputs(**inputs)
    last_exc = None
    for attempt in range(3):
        try:
            r = bass_utils.run_bass_kernel_spmd(
                nc, in_maps, core_ids=list(range(NCORES)), trace=trace)
            break
        except Exception as exc:  # rare transient NRT exec-unit failures
            last_exc = exc
            import jax
            try:
                jax.clear_caches()
            except Exception:
                pass
            try:
                from concourse import bass2jax
                bass2jax._bass_exec_cache.clear()
            except Exception:
                pass
            _STATE.clear()
            nc = _get_nc()
    else:
        raise last_exc
    # Reduce-scatter ops cover [b0, b1, b2, hb6, hb7]; core c owns the
    # c-th shard of each op's token range.
    RSQ = [256, 256, 256, 128, 128]
    RSTB = [0, 2048, 4096, 6144, 7168]
    RSC0 = [0, 256, 512, 768, 896]
    full = np.empty((B * T, DV), np.float32)
    for c in range(NCORES):
        oc = np.asarray(r.results[c]["out"]).astype(np.float32).T
        for i in range(len(RSQ)):
            q = RSQ[i]
            full[RSTB[i] + q * c: RSTB[i] + q * (c + 1)] = \
                oc[RSC0[i]: RSC0[i] + q]
    full = full.reshape(B, T, DV)
    return full, r


def kernel(**inputs):
    full, _ = _run(inputs, trace=False)
    return full


# revision 83
# speedup vs baseline: 1.0305x; 1.0305x over previous
"""Trainium2 Bass kernel for conv-QK causal attention + MLP.

Reference computation (B=4, T=2048, D=512, H=8, DK=DV=64, FS=3):
  q = causal_conv1d(x, Wq) + bq ; k = causal_conv1d(x, Wk) + bk
  v = x @ Wv + bv
  per-head causal attention (softmax(q k^T / 8))
  out = relu(attn @ W1 + b1) @ W2 + b2        -> [B, T, 64]

Sharding: head-parallel, one head per NeuronCore (H == 8 == n_cores).
Each core computes q/k/v and attention for its head over all batches,
its partial attn @ W1[head], then ReduceScatters to sum head partials
and shard tokens 8-way for the final relu/W2 epilogue.

On-chip layout is "transposed" (channels on partitions, tokens on the
free axis) so softmax needs no transposes at all:
  St[k, q] = K Q^T per 128-row k-block strip; exp on ScalarE (no max
  subtraction: logits are O(1) by construction); causal masking via a
  0/1 upper-triangular multiply on the diagonal block; P V done as
  O~^T = [V | 1]^T P^T which also accumulates the softmax denominators
  as row 64 of the PSUM accumulator; softmax normalization happens
  AFTER the W1 partial (1/l applied to the W1 output on DVE).

Key perf structure (v2), driven by the HAM clock-gate trace: the PE
HAM demotes to K=4/8 (1.2 GHz) during stretches dominated by thin
64-row stationaries and any >3.4us PE idle window.  So:
  - Batch b+1's projection work (QK conv, V, V-transposes — all
    128-row stationaries) is interleaved step-by-step into batch b's
    attention phase: the PE stream stays dense and majority-full-width
    through the whole kernel, and conv/attention no longer alternate
    as ACT-bound vs PE-bound phases.
  - QK^T strips use a 128-row stationary kzt whose rows 64:128 are
    zeros (K^T lives in rows 0:64); the moving operand is the full
    128-row qkt tile (rows 64:128 are K^T, killed by the zero rows).
    Mathematically identical, but the stationary is full-height.
  - V^T staging tiles are 128 rows (rows 65:128 zeroed) so the PE
    transposes are full-height too.
  - QK^T emits per-512-column chunks into single-bank PSUM tiles
    (4-buf rotation) with exp issued per chunk: finer ACT pipelining
    and earlier PSUM release.
  - Collectives: the reduce-scatter stream is serial (~10-20 GB/s per
    op + ~6us trigger overhead, each op's duration also absorbing
    cross-core skew), so the payload is bf16 and grouped as
    [b0, b1, b2, b3h0, b3h1] — the middle ops drain while compute
    runs and only a small 128-token-shard op sits in the kernel tail.
    The FIRST collective's trigger would wait for the startup barrier
    on the gpsimd queue (stalling the close-chain partition_broadcasts
    behind it, measured -22us), so it is deferred a few iterations.
  - The epilogue is split: batches 0-2 in one fused pass that overlaps
    the compute drain, then each batch-3 half as its own small chain
    the moment its collective lands.

dtypes: the whole heavy path runs in bf16; PSUM accumulation and the
softmax denominators stay fp32.  The PE queue is strictly in-order, so
anything that waits on ACT/DVE/collective latency is emitted a few
pair-iterations after the data chain that feeds it (the `pending`
deferral list) or it stalls every matmul behind it.
"""

import ml_dtypes
import numpy as np

import concourse.bass as bass
import concourse.mybir as mybir
import concourse.tile as tile
from concourse import bacc, bass_utils
from concourse.masks import make_identity, make_upper_triangular

B, T, D = 4, 2048, 512
H, DK, DV, FS = 8, 64, 64, 3
NCORES = 8
TP = T + FS - 1          # left-zero-padded time axis (2050)
NDT = D // 128           # d-tiles (4)
NTT = T // 128           # t-tiles (16)
TOK = B * T // NCORES    # 1024 output tokens per core
NCH = T // 512           # 512-col conv chunks per batch (4)

F32 = mybir.dt.float32
BF16 = mybir.dt.bfloat16

_STATE = {}


def _build():
    nc = bacc.Bacc("TRN2", target_bir_lowering=False, debug=False,
                   num_devices=NCORES)

    xtp = nc.dram_tensor("xtp", [B, D, TP], BF16, kind="ExternalInput")
    wqk = nc.dram_tensor("wqk", [FS, NDT, 128, 128], BF16, kind="ExternalInput")
    wv = nc.dram_tensor("wv", [NDT, 128, DV], BF16, kind="ExternalInput")
    w1 = nc.dram_tensor("w1", [DV, 64], BF16, kind="ExternalInput")
    w2 = nc.dram_tensor("w2", [64, 64], BF16, kind="ExternalInput")
    bqk = nc.dram_tensor("bqk", [128, 1], F32, kind="ExternalInput")
    bv = nc.dram_tensor("bv", [64, 1], F32, kind="ExternalInput")
    b1 = nc.dram_tensor("b1", [64, 1], F32, kind="ExternalInput")
    b2 = nc.dram_tensor("b2", [64, 1], F32, kind="ExternalInput")
    out = nc.dram_tensor("out", [DV, TOK], BF16, kind="ExternalOutput")

    EXP = mybir.ActivationFunctionType.Exp

    with tile.TileContext(nc) as tc:
        with (
            tc.tile_pool(name="cpool", bufs=1) as cpool,
            tc.tile_pool(name="xpool", bufs=2) as xpool,
            tc.tile_pool(name="qkpool", bufs=2) as qkpool,
            tc.tile_pool(name="vpool", bufs=2) as vpool,
            tc.tile_pool(name="ptpool", bufs=6) as ptpool,
            tc.tile_pool(name="atpool", bufs=2) as atpool,
            tc.tile_pool(name="spool", bufs=2) as spool,
            tc.tile_pool(name="stpool", bufs=4, space="PSUM") as stpool,
            tc.tile_pool(name="opool", bufs=2, space="PSUM") as opool,
            tc.tile_pool(name="cvpool", bufs=2, space="PSUM") as cvpool,
            tc.tile_pool(name="dpool", bufs=1, space="DRAM") as dpool,
        ):
            xtp_tiles = {}

            def load_xtp(b):
                # one tile PER d-quarter: readers then depend on exactly
                # the quarter's DMA, not conservatively on all four
                # sequential ring transfers (~13us for the whole load)
                src = xtp.ap()[b].rearrange("(dt p) t -> p dt t", p=128)
                qs_ = []
                for dt_ in range(NDT):
                    t_ = xpool.tile([128, TP], BF16,
                                    name=f"xtp_sb{b}_{dt_}",
                                    tag=f"xtp{dt_}")
                    # scalar-engine hwdge ring: keeps the 525KB x loads
                    # off the sync ring (rs_in/zin/consts traffic)
                    nc.scalar.dma_start(t_[:], src[:, dt_])
                    qs_.append(t_)
                xtp_tiles[b] = qs_

            # ---- wqk then x batch 0 first (warmup needs wqk, the first
            # conv matmuls need both); everything else loads behind them ----
            wqk_sb = cpool.tile([128, FS, NDT, 128], BF16)
            nc.sync.dma_start(wqk_sb[:], wqk.ap().rearrange("f dt p m -> p f dt m"))
            load_xtp(0)
            _deferred_consts = []
            _orig_dma = nc.sync.dma_start

            def _defer_dma(*a, **k):
                _deferred_consts.append((a, k))
            wv_sb = cpool.tile([128, NDT, DV], BF16)
            _defer_dma(wv_sb[:], wv.ap().rearrange("dt p m -> p dt m"))
            w1_sb = cpool.tile([DV, 64], BF16)
            _defer_dma(w1_sb[:], w1.ap())
            w2_sb = cpool.tile([64, 64], BF16)
            _defer_dma(w2_sb[:], w2.ap())
            bqk_sb = cpool.tile([128, 1], F32)
            _defer_dma(bqk_sb[:], bqk.ap())
            bv_sb = cpool.tile([64, 1], F32)
            _defer_dma(bv_sb[:], bv.ap())
            b1_sb = cpool.tile([64, 1], F32)
            _defer_dma(b1_sb[:], b1.ap())
            b2_sb = cpool.tile([64, 1], F32)
            _defer_dma(b2_sb[:], b2.ap())
            maskf_sb = cpool.tile([128, 128], F32)
            make_upper_triangular(nc, maskf_sb[:], val=1.0, diag=True)
            mask_sb = cpool.tile([128, 128], BF16)
            nc.vector.tensor_copy(mask_sb[:], maskf_sb[:])
            # touch Exp once so the ~2.7us ACT table load happens during
            # startup instead of stalling the first attention strip
            junk_sb = cpool.tile([1, 128], F32)
            nc.scalar.activation(junk_sb[:], maskf_sb[0:1, :],
                                 mybir.ActivationFunctionType.Exp, scale=1.0)
            # persistent V^T staging tiles: 128 rows so the PE transposes
            # are full-height; row 64 is the ones row ([V | 1] denominator
            # column), rows 65:128 are zeros
            vt_tiles = [cpool.tile([128, T], BF16, name=f"vt_t{i}")
                        for i in range(2)]
            for t_ in vt_tiles:
                nc.gpsimd.memset(t_[64:128, :].bitcast(mybir.dt.uint16), 0)
                nc.gpsimd.memset(t_[64:65, :].bitcast(mybir.dt.uint16),
                                 0x3F80)  # bf16 1.0
            # persistent K^T stationary tiles: rows 0:64 = K^T (written per
            # batch by the conv drains), rows 64:128 = zeros, so QK^T loads
            # a full 128-row stationary (HAM stays warm) while the junk
            # K^T rows of the moving qkt operand are multiplied by zero
            kzt_tiles = [cpool.tile([128, T], BF16, name=f"kzt_t{i}")
                         for i in range(2)]
            for t_ in kzt_tiles:
                nc.gpsimd.memset(t_[64:128, :].bitcast(mybir.dt.uint16), 0)
            identf_sb = cpool.tile([128, 128], F32)
            make_identity(nc, identf_sb[:])
            ident_sb = cpool.tile([128, 128], BF16)
            nc.vector.tensor_copy(ident_sb[:], identf_sb[:])

            # The collective stream is serial, ~10-20 GB/s per op plus
            # ~6us trigger overhead, and can only start once the first
            # op's inputs exist — so lead with two small half-batch ops
            # (ready earliest), batch the middle, and end with two small
            # ops so only ~9us of collective sits in the kernel tail.
            RSQ = [256, 256, 256, 128, 128]  # shard tokens per op
            RSTB = [0, 2048, 4096, 6144, 7168]  # first token per op
            NRS = len(RSQ)
            rs_in = [dpool.tile([NCORES, 64, RSQ[i]], BF16, name=f"rs_in{i}",
                                tag=f"rsi{i}", uniquify=False)
                     for i in range(NRS)]
            rs_out = [dpool.tile([64, RSQ[i]], BF16, name=f"rs_out{i}",
                                 tag=f"rso{i}", uniquify=False)
                      for i in range(NRS)]
            rs_left = {0: 4, 1: 4, 2: 4, 3: 2, 4: 2}

            # The PE queue is strictly in-order, so a matmul that depends on
            # the (ACT/DMA/DVE) softmax-normalization chain must be emitted a
            # couple of pair-iterations later than the chain or it stalls the
            # whole queue.  pending holds [age, closure, threshold] entries.
            pending = []

            def flush_pending(extra_age):
                keep = []
                for item in pending:
                    if item[0] + extra_age >= item[2]:
                        item[1]()
                    else:
                        keep.append(item)
                pending[:] = keep

            def age_pending():
                for item in pending:
                    item[0] += 1

            # ---- projection work for batch b, as a list of fine-grained
            # steps (~0.5-1us of PE work each) interleaved into batch b-1's
            # attention phase.  All stationaries are full 128-row. ----
            prep = {}

            def make_conv_steps(b):
                xsb = xtp_tiles.pop(b)
                qkt = qkpool.tile([128, T], BF16, name=f"qkt{b}", tag="qkt")
                kzt = kzt_tiles[b % 2]
                vt = vt_tiles[b % 2]
                vsb = vpool.tile([128, NTT, 80], BF16, name=f"v_sb{b}",
                                 tag="v")
                prep[b] = (qkt, vsb)
                steps = []

                # chunk PAIRS share each stationary (consecutive matmuls
                # elide the redundant LDWEIGHTS: 24 loads/batch not 48);
                # dt outer so the first steps only need the first xtp
                # quarter (the conv can start while the rest still loads)
                def qk_step(cp, dt_, f, qkcs):
                    def run():
                        for ci in range(2):
                            c = 2 * cp + ci
                            nc.tensor.matmul(
                                qkcs[ci][:],
                                wqk_sb[:, f, dt_, :],
                                xsb[dt_][:, c * 512 + f: c * 512 + f + 512],
                                start=(dt_ == 0 and f == 0),
                                stop=(dt_ == NDT - 1 and f == FS - 1))
                        if dt_ == NDT - 1 and f == FS - 1:
                            for ci in range(2):
                                sl = bass.ts(2 * cp + ci, 512)
                                nc.vector.tensor_scalar_add(
                                    qkt[:, sl], qkcs[ci][:], bqk_sb[:])
                                nc.vector.tensor_copy(
                                    kzt[0:64, sl], qkt[64:128, sl])
                    return run

                for cp in range(NCH // 2):
                    qkcs = [cvpool.tile([128, 512], F32, tag="cv",
                                        name=f"qkc{b}_{2 * cp + ci}")
                            for ci in range(2)]
                    for dt_ in range(NDT):
                        for f in range(FS):
                            steps.append(qk_step(cp, dt_, f, qkcs))

                def v_step(cp, dt_, vcs):
                    def run():
                        for ci in range(2):
                            c = 2 * cp + ci
                            nc.tensor.matmul(
                                vcs[ci][0:64, :],
                                wv_sb[:, dt_, :],
                                xsb[dt_][:, c * 512 + 2: c * 512 + 2 + 512],
                                start=(dt_ == 0), stop=(dt_ == NDT - 1))
                        if dt_ == NDT - 1:
                            for ci in range(2):
                                nc.vector.tensor_scalar_add(
                                    vt[0:64, bass.ts(2 * cp + ci, 512)],
                                    vcs[ci][0:64, :], bv_sb[:])
                    return run

                def tr_step(c):
                    def run():
                        for tt in range(4 * c, 4 * c + 4):
                            trp = cvpool.tile([128, 128], BF16, tag="cv",
                                              name=f"trp{b}_{tt}")
                            nc.tensor.transpose(
                                trp[:, 0:80], vt[:, bass.ts(tt, 128)],
                                ident_sb[:, 0:80])
                            nc.vector.tensor_copy(vsb[:, tt, :],
                                                  trp[:, 0:80])
                    return run

                for cp in range(NCH // 2):
                    vcs = [cvpool.tile([128, 512], F32, tag="cv",
                                       name=f"vc{b}_{2 * cp + ci}")
                           for ci in range(2)]
                    for dt_ in range(NDT):
                        steps.append(v_step(cp, dt_, vcs))
                    steps.append(tr_step(2 * cp))
                    steps.append(tr_step(2 * cp + 1))
                return steps

            conv_steps = []

            def take_conv(iters_left):
                if not conv_steps:
                    return
                n = -(-len(conv_steps) // max(iters_left, 1))
                for _ in range(min(n, len(conv_steps))):
                    conv_steps.pop(0)()

            # ---- PE warmup while the first DMAs land ----
            warm_ps = stpool.tile([128, 512], F32, tag="st")
            for wi in range(8):
                nc.tensor.matmul(
                    warm_ps[:], wqk_sb[:, 0, 0, :], wqk_sb[:, 0, :, 0:128],
                    start=(wi == 0), stop=(wi == 7))
            for a_, k_ in _deferred_consts:
                _orig_dma(*a_, **k_)
            _deferred_consts.clear()
            load_xtp(1)
            # the warm-result DMA waits on the warmup matmuls, so it goes
            # last in the queue (nothing behind it to block)
            warm_sb = cpool.tile([1, 1], F32)
            nc.vector.tensor_copy(warm_sb[:], warm_ps[0:1, 0:1])
            nc.sync.dma_start(out.ap()[0:1, 0:2].bitcast(F32), warm_sb[:])

            # batch 0's projections run as a dense phase (nothing to
            # interleave them into)
            for s in make_conv_steps(0):
                s()

            for b in range(B):
                qkt_sb, v_sb = prep.pop(b)
                kzt = kzt_tiles[b % 2]
                if b + 2 <= B - 1:
                    load_xtp(b + 2)
                if b + 1 <= B - 1:
                    conv_steps.extend(make_conv_steps(b + 1))
                # pair-iterations over both passes: 4+2 + 8+2
                iters_left = (1024 // 256 + 2) + (2048 // 256 + 2)

                # ---- attention, two q-passes of 1024 columns ----
                attnT_sb = atpool.tile([64, T], BF16, tag="attnT")
                # bf16: the reduce-scatter payload halves (the collective
                # stream is the kernel-tail critical path)
                p1_sb = atpool.tile([64, T], BF16, tag="p1")
                for ps in (0, 1):
                    qlo, qhi = ps * 1024, ps * 1024 + 1024
                    o_ps = [opool.tile([128, 512], F32, tag="o",
                                       name=f"o_ps_{b}_{ps}_{c}")
                            for c in range(2)]
                    nkb = qhi // 128

                    def do_pv(strips, o_ps=o_ps, qlo=qlo, b=b):
                        """PV matmuls for a pair of exp'd strips; returns the
                        chunks whose accumulation closed."""
                        closed = []
                        for kb, qs, w, pt in strips:
                            for qc in range(2):
                                clo = qlo + qc * 512
                                chi = clo + 512
                                lo = max(clo, qs)
                                n = chi - lo
                                if n <= 0:
                                    continue
                                stop = kb == chi // 128 - 1
                                nc.tensor.matmul(
                                    o_ps[qc][0:DV + 1, lo - clo: lo - clo + n],
                                    v_sb[:, kb, 0:DV + 1],
                                    pt[:, lo - qs: lo - qs + n],
                                    start=(kb == 0),
                                    stop=stop)
                                if stop:
                                    closed.append(qc)
                        return closed

                    window = []  # strip pairs awaiting PV (lag 2)
                    for kba in list(range(0, nkb, 2)) + [None, None]:
                        take_conv(iters_left)
                        iters_left -= 1
                        age_pending()
                        flush_pending(0)
                        strips = []
                        if kba is not None:
                            for kb in (kba, kba + 1):
                                qs = max(qlo, kb * 128)
                                w = qhi - qs
                                pt = ptpool.tile([128, 1024], BF16, tag="pt")
                                lhsT = kzt[:, bass.ts(kb, 128)]
                                for c0 in range(0, w, 512):
                                    cw = min(512, w - c0)
                                    stc = stpool.tile([128, 512], F32,
                                                      tag="st")
                                    nc.tensor.matmul(
                                        stc[:, 0:cw], lhsT,
                                        qkt_sb[:, qs + c0: qs + c0 + cw],
                                        start=True, stop=True)
                                    nc.scalar.activation(
                                        pt[:, c0:c0 + cw], stc[:, 0:cw],
                                        EXP, scale=0.125)
                                    if c0 == 0 and kb * 128 >= qlo:
                                        nc.vector.tensor_mul(
                                            pt[:, 0:128], pt[:, 0:128],
                                            mask_sb[:])
                                strips.append((kb, qs, w, pt))
                        # PV lags two pair-iterations so it never waits on exp
                        if strips:
                            window.append(strips)
                        closed = []
                        if len(window) == 3 or (kba is None and window):
                            closed = do_pv(window.pop(0))
                        # per-chunk tail as soon as its accumulation closes:
                        # 1/l straight off the PSUM denominator row, copy the
                        # unnormalized rows to bf16, then (deferred) W1 into
                        # partitions 64:128 of the same bank and normalize on
                        # the way out to the reduce-scatter buffer.
                        for qc in closed:
                            clo = qlo + qc * 512
                            o_t = o_ps[qc]
                            l_sb = spool.tile([1, 512], F32, tag="l")
                            nc.vector.tensor_copy(l_sb[:], o_t[DV:DV + 1, :])
                            linv_sb = spool.tile([1, 512], F32, tag="linv")
                            nc.vector.reciprocal_approx_fast(
                                linv_sb[:], l_sb[:])
                            nc.vector.tensor_copy(
                                attnT_sb[:, clo:clo + 512], o_t[0:DV, :])

                            lbc_sb = spool.tile([64, 512], F32, tag="lbc")
                            nc.gpsimd.partition_broadcast(lbc_sb[:], linv_sb[:])

                            def p1_work(clo=clo, b=b, o_t=o_t, lbc_sb=lbc_sb,
                                        attnT_sb=attnT_sb, p1_sb=p1_sb):
                                nc.tensor.matmul(
                                    o_t[64:128, :], w1_sb[:],
                                    attnT_sb[:, clo:clo + 512],
                                    start=True, stop=True)
                                nc.vector.tensor_mul(
                                    p1_sb[:, clo:clo + 512], o_t[64:128, :],
                                    lbc_sb[:])
                                idx = b if b < B - 1 else 3 + clo // 1024
                                qsh = RSQ[idx]
                                for hh in range(512 // qsh):
                                    s0 = clo + qsh * hh
                                    nc.sync.dma_start(
                                        rs_in[idx][
                                            (s0 % (NCORES * qsh)) // qsh],
                                        p1_sb[:, s0:s0 + qsh])
                                rs_left[idx] -= 1
                                if rs_left[idx] == 0:
                                    def rs_fire(idx=idx):
                                        nc.gpsimd.collective_compute(
                                            "ReduceScatter",
                                            mybir.AluOpType.add,
                                            replica_groups=[
                                                list(range(NCORES))],
                                            ins=[rs_in[idx].opt()],
                                            outs=[rs_out[idx].opt()],
                                        )
                                    if idx == 0:
                                        # the first collective's trigger
                                        # waits for the startup barrier ON
                                        # THE GPSIMD QUEUE, stalling the
                                        # close-chain broadcasts behind it
                                        # — fire it a few iterations late
                                        pending.append([0, rs_fire, 6])
                                    else:
                                        rs_fire()
                            pending.append([0, p1_work, 2])
                # any projection steps not consumed by the pair-iterations
                while conv_steps:
                    conv_steps.pop(0)()

            flush_pending(99)

            # ---- split epilogue: relu(z + b1) @ W2 + b2.  The batch-0..2
            # part overlaps the compute drain (its collectives finish while
            # batch 3 computes); each batch-3 half runs as soon as its own
            # collective lands.  out columns: [b0|b1|b2|b3h0|b3h1]. ----
            RSC0 = [0, 256, 512, 768, 896]  # out column start per op
            zin_sb = spool.tile([64, 1024], BF16)
            z_sb = spool.tile([64, 1024], BF16)
            fin_sb = spool.tile([64, 1024], BF16)

            def epi_part(ops):
                c0, c1 = RSC0[ops[0]], RSC0[ops[-1]] + RSQ[ops[-1]]
                for q in ops:
                    nc.sync.dma_start(
                        zin_sb[:, bass.ds(RSC0[q], RSQ[q])], rs_out[q][:])
                sl = bass.ds(c0, c1 - c0)
                nc.vector.tensor_scalar(
                    z_sb[:, sl], zin_sb[:, sl], b1_sb[:], 0.0,
                    op0=mybir.AluOpType.add, op1=mybir.AluOpType.max)
                for m0 in range(c0, c1, 512):
                    cw = min(512, c1 - m0)
                    f_ps = opool.tile([64, 512], F32, tag="o",
                                      name=f"f_ps{m0}")
                    nc.tensor.matmul(f_ps[0:64, 0:cw], w2_sb[:],
                                     z_sb[:, m0:m0 + cw],
                                     start=True, stop=True)
                    nc.vector.tensor_scalar_add(
                        fin_sb[:, m0:m0 + cw], f_ps[0:64, 0:cw], b2_sb[:])
                nc.sync.dma_start(out.ap()[:, sl], fin_sb[:, sl])

            epi_part([0, 1, 2])
            epi_part([3])
            epi_part([4])

    nc.compile()
    return nc


def _get_nc():
    if "nc" not in _STATE:
        _STATE["nc"] = _build()
    return _STATE["nc"]


def _prep_inputs(x, Wq, bq, Wk, bk, Wv, bv, W1, b1, W2, b2):
    f = np.float32
    x = np.ascontiguousarray(np.asarray(x, f))
    xtp = np.zeros((B, D, TP), f)
    xtp[:, :, FS - 1:] = x.transpose(0, 2, 1)
    Wq = np.asarray(Wq, f)
    Wk = np.asarray(Wk, f)
    Wv = np.asarray(Wv, f)
    W1 = np.asarray(W1, f)
    W2 = np.asarray(W2, f)
    bq = np.asarray(bq, f)
    bk = np.asarray(bk, f)
    bv = np.asarray(bv, f)
    b1 = np.asarray(b1, f)
    b2 = np.asarray(b2, f)

    bf = ml_dtypes.bfloat16
    xtp = np.ascontiguousarray(xtp.astype(bf))
    in_maps = []
    for c in range(NCORES):
        hs = slice(c * DK, (c + 1) * DK)
        wqk_c = np.concatenate([Wq[:, :, hs], Wk[:, :, hs]], axis=2)
        in_maps.append({
            "xtp": xtp,
            "wqk": np.ascontiguousarray(
                wqk_c.reshape(FS, NDT, 128, 128).astype(bf)),
            "wv": np.ascontiguousarray(Wv[:, hs].reshape(NDT, 128, DV).astype(bf)),
            "w1": np.ascontiguousarray(W1[hs, :].astype(bf)),
            "w2": np.ascontiguousarray(W2.astype(bf)),
            "bqk": np.ascontiguousarray(
                np.concatenate([bq[hs], bk[hs]])[:, None]),
            "bv": np.ascontiguousarray(bv[hs][:, None]),
            "b1": np.ascontiguousarray(b1[:, None]),
            "b2": np.ascontiguousarray(b2[:, None]),
        })
    return in_maps


def _run(inputs, trace=False):
    nc = _get_nc()
    in_maps = _prep_inputs(**inputs)
    last_exc = None
    for attempt in range(3):
        try:
            r = bass_utils.run_bass_kernel_spmd(
                nc, in_maps, core_ids=list(range(NCORES)), trace=trace)
            break
        except Exception as exc:  # rare transient NRT exec-unit failures
            last_exc = exc
            import jax
            try:
                jax.clear_caches()
            except Exception:
                pass
            try:
                from concourse import bass2jax
                bass2jax._bass_exec_cache.clear()
            except Exception:
                pass
            _STATE.clear()
            nc = _get_nc()
    else:
        raise last_exc
    # Reduce-scatter ops cover [b0, b1, b2, hb6, hb7]; core c owns the
    # c-th shard of each op's token range.
    RSQ = [256, 256, 256, 128, 128]
    RSTB = [0, 2048, 4096, 6144, 7168]
    RSC0 = [0, 256, 512, 768, 896]
    full = np.empty((B * T, DV), np.float32)
    for c in range(NCORES):
        oc = np.asarray(r.results[c]["out"]).astype(np.float32).T
        for i in range(len(RSQ)):
            q = RSQ[i]
            full[RSTB[i] + q * c: RSTB[i] + q * (c + 1)] = \
                oc[RSC0[i]: RSC0[i] + q]
    full = full.reshape(B, T, DV)
    return full, r


def kernel(**inputs):
    full, _ = _run(inputs, trace=False)
    return full
